# revision 1
# baseline (speedup 1.0000x reference)
"""DINOv3 ViT attention (RoPE + det-temp scaling + additive gate) on 8 TRN2 cores.

Sharding: pure data-parallel over batch (B=8 -> 1 batch element per core).
Weights / gate / rope tables replicated. No collectives.

Per-core algorithm (all matmuls fp32r = fp32 rounded to 11 mantissa bits,
1 PE cycle/column; S padded 1129->1152):
  phase 1: qT/kT [dout,s] and v [s,dout] projections from host-transposed
           hsT/weights; v bias via a K=1 ones-row matmul; q bias + 1/sqrt(hd)
           folded into the ACT eviction.
  phase 2: RoPE: rotate_half via a PE permutation matmul, then 3 DVE ops
           against stacked cos/sin tables; det-temp scaling via a PSUM
           outer-product pattern tile; fused per dout-tile, in place.
  phase 3: scoresT[sk,sq] = gate^T (identity-matmul copy into PSUM) +
           kT^T q (K=64, head pairs on disjoint 64-row PE strips via
           tile_position so they run concurrently); exp on ACT (no max
           subtraction -- scores are O(1) by construction); ctxT[hd+1,sq]
           accumulated over sk with a ones column in v producing the softmax
           denominator; normalization = DVE reciprocal of the denominator row
           + PE outer-product broadcast + DVE multiply; output projection
           accumulated over ctxT tiles + o_b added at eviction.

The harness contract: kernel(**inputs) with FULL inputs, returns FULL output.
"""
import numpy as np
from contextlib import ExitStack

import concourse.bacc as bacc
import concourse.mybir as mybir
import concourse.tile as tile
from concourse.bass_utils import run_bass_kernel_spmd

F32 = mybir.dt.float32
F32R = mybir.dt.float32r
AF = mybir.ActivationFunctionType

# ---------------- problem config (hardcoded per harness contract) ------------


class CFG:
    B = 8
    S = 1129
    SP = 1152            # padded S (9 * 128)
    D = 768
    H = 12
    HD = 64
    ROPE_START = 5
    ROPE_END = 1029
    DET_START = 1029
    DET_END = 1129
    P_SCALE = 2.0
    N_CORES = 8
    SQB = 384            # sq block (>=256 keeps fp32r at 1 cyc/row; 3 | SP)
    GATE_NEG = -30.0     # gate value for pad keys: exp(-30) ~ 9e-14
    ONLY_PHASE1 = False

    @property
    def KT(self):
        return self.D // 128          # dout/din 128-tiles (6)

    @property
    def NT(self):
        return self.SP // 128         # s 128-tiles (9)

    @property
    def NB(self):
        return self.SP // self.SQB    # sq blocks (3)

    @property
    def ROPE_LEN(self):
        return self.ROPE_END - self.ROPE_START


def round_f32r(x: np.ndarray) -> np.ndarray:
    """Round fp32 to the fp32r format (11 mantissa bits, RNE)."""
    b = np.ascontiguousarray(x, dtype=np.float32).view(np.uint32)
    low = b & np.uint32(0xFFF)
    b = b & np.uint32(0xFFFFF000)
    rnd = (low > 0x800) | ((low == 0x800) & (((b >> 12) & 1) != 0))
    b = b + (rnd.astype(np.uint32) << 12)
    return b.view(np.float32)


# ---------------- device program ------------------------------------------


def build_nc(cfg: CFG):
    nc = bacc.Bacc(trn_type="TRN2", target_bir_lowering=False, debug=False)
    KT, NT, NB, SQB, SP = cfg.KT, cfg.NT, cfg.NB, cfg.SQB, cfg.SP
    H, HD = cfg.H, cfg.HD
    RS, RE, DS, DE = cfg.ROPE_START, cfg.ROPE_END, cfg.DET_START, cfg.DET_END
    RL = cfg.ROPE_LEN
    DET = DE - DS

    # ---- dram parameters (per core) ----
    d_hsT = nc.dram_tensor("hsT", [cfg.D, SP], F32R, kind="ExternalInput").ap()
    d_qwT = nc.dram_tensor("qwT", [cfg.D, cfg.D], F32R, kind="ExternalInput").ap()
    d_kwT = nc.dram_tensor("kwT", [cfg.D, cfg.D], F32R, kind="ExternalInput").ap()
    d_vwT = nc.dram_tensor("vwT", [(KT + 1) * 128, cfg.D], F32R, kind="ExternalInput").ap()
    d_owT = nc.dram_tensor("owT", [cfg.D, cfg.D], F32R, kind="ExternalInput").ap()
    d_gateT = nc.dram_tensor("gateT", [SP, SP], F32R, kind="ExternalInput").ap()
    d_qb = nc.dram_tensor("qb", [128, KT], F32, kind="ExternalInput").ap()
    d_ob = nc.dram_tensor("ob", [128, cfg.D], F32, kind="ExternalInput").ap()
    d_cosT2 = nc.dram_tensor("cosT2", [128, RL], F32, kind="ExternalInput").ap()
    d_sinT2 = nc.dram_tensor("sinT2", [128, RL], F32, kind="ExternalInput").ap()
    d_rotT = nc.dram_tensor("rotT", [128, 128], F32R, kind="ExternalInput").ap()
    d_ident = nc.dram_tensor("ident", [128, 128], F32R, kind="ExternalInput").ap()
    d_ones65 = nc.dram_tensor("ones65", [65, 128], F32R, kind="ExternalInput").ap()
    d_onescol = nc.dram_tensor("onescol", [128, H], F32R, kind="ExternalInput").ap()
    d_masks = nc.dram_tensor("masks", [1, 256], F32R, kind="ExternalInput").ap()
    d_ph = nc.dram_tensor("ph", [1, DET], F32, kind="ExternalInput").ap()
    d_pw = nc.dram_tensor("pw", [1, DET], F32, kind="ExternalInput").ap()
    d_out = nc.dram_tensor("out", [SP, cfg.D], F32, kind="ExternalOutput").ap()

    with tile.TileContext(nc) as tc, ExitStack() as gctx:
        # ---------------- global pools (span the whole kernel) --------------
        gsb = gctx.enter_context(tc.tile_pool(name="gsb", bufs=1))

        # small constants
        t_ident = gsb.tile([128, 128], F32R, tag="ident")
        nc.sync.dma_start(t_ident[:], d_ident[:, :])
        t_ones65 = gsb.tile([65, 128], F32R, tag="ones65")
        nc.sync.dma_start(t_ones65[:], d_ones65[:, :])


        # gate tiles live in the global pool; DMAs are emitted later (after the
        # phase-1 critical loads) so they don't delay hsT/weights at startup
        t_gate = [gsb.tile([128, SP], F32R, tag=f"g{t}", name=f"g{t}")
                  for t in range(NT)]

        # persistent activation tensors
        t_qTf = [gsb.tile([128, SP], F32R, tag=f"qTf{m}", name=f"qTf{m}") for m in range(KT)]
        t_kTf = [gsb.tile([128, SP], F32R, tag=f"kTf{m}", name=f"kTf{m}") for m in range(KT)]
        t_v = [gsb.tile([128, H * 65], F32R, tag=f"v{t}", name=f"v{t}") for t in range(NT)]

        # ==================== phase 1 + 2: projections & q/k finalize =======
        with ExitStack() as p1:
            sb1 = p1.enter_context(tc.tile_pool(name="sb1", bufs=1))
            wsb = p1.enter_context(tc.tile_pool(name="wsb", bufs=2))
            tsb = p1.enter_context(tc.tile_pool(name="tsb", bufs=1))
            ps_q = p1.enter_context(tc.tile_pool(name="ps_q", bufs=2, space="PSUM"))
            ps_big = p1.enter_context(tc.tile_pool(name="ps_big", bufs=2, space="PSUM"))
            ps_tem = p1.enter_context(tc.tile_pool(name="ps_tem", bufs=1, space="PSUM"))

            # hsT tiles interleaved with qw loads so the first q chain can
            # start accumulating as tiles arrive
            t_hsT = []
            qw = []
            for k in range(KT):
                t = sb1.tile([128, SP], F32R, tag=f"hsT{k}", name=f"hsT{k}")
                eng = nc.sync if k % 2 == 0 else nc.scalar
                eng.dma_start(t[:], d_hsT[k * 128:(k + 1) * 128, :])
                t_hsT.append(t)
                w = wsb.tile([128, cfg.D], F32R, tag=f"w{k}", name=f"qw{k}")
                weng = nc.scalar if k % 2 == 0 else nc.sync
                weng.dma_start(w[:], d_qwT[k * 128:(k + 1) * 128, :])
                qw.append(w)


            # rope/det tables
            t_cos = sb1.tile([128, RL], F32, tag="cos")
            nc.scalar.dma_start(t_cos[:], d_cosT2[:, :])
            t_sin = sb1.tile([128, RL], F32, tag="sin")
            nc.scalar.dma_start(t_sin[:], d_sinT2[:, :])
            t_rotT = sb1.tile([128, 128], F32R, tag="rotT")
            nc.sync.dma_start(t_rotT[:], d_rotT[:, :])
            t_qb = sb1.tile([128, KT], F32, tag="qb")
            nc.sync.dma_start(t_qb[:], d_qb[:, :])

            # det temperature pattern tile: [128, DET] via two outer products
            t_ms = sb1.tile([1, 256], F32R, tag="ms")
            nc.sync.dma_start(t_ms[:], d_masks[:, :])
            t_ph = sb1.tile([1, DET], F32, tag="ph")
            nc.sync.dma_start(t_ph[:], d_ph[:, :])
            t_pw = sb1.tile([1, DET], F32, tag="pw")
            nc.sync.dma_start(t_pw[:], d_pw[:, :])
            t_eh = sb1.tile([1, DET], F32R, tag="eh")
            nc.scalar.activation(t_eh[:], t_ph[:], AF.Exp, bias=0.0, scale=cfg.P_SCALE)
            t_ew = sb1.tile([1, DET], F32R, tag="ew")
            nc.scalar.activation(t_ew[:], t_pw[:], AF.Exp, bias=0.0, scale=cfg.P_SCALE)
            p_tem = ps_tem.tile([128, DET], F32, tag="tem")
            nc.tensor.matmul(p_tem[:], t_ms[0:1, 0:128], t_eh[:], start=True, stop=False)
            nc.tensor.matmul(p_tem[:], t_ms[0:1, 128:256], t_ew[:], start=False, stop=True)

            def load_w(dram, k):
                t = wsb.tile([128, cfg.D], F32R, tag=f"w{k}")
                nc.sync.dma_start(t[:], dram[k * 128:(k + 1) * 128, :])
                return t

            def finalize_qk(raw, dst):
                """RoPE + det-temp + prefix/tail copy: raw [128,SP] F32 -> dst F32R."""
                p_rot = ps_big.tile([128, RL], F32, tag="big")
                for c0 in range(0, RL, 512):
                    cw = min(512, RL - c0)
                    nc.tensor.matmul(p_rot[:, c0:c0 + cw], t_rotT[:],
                                     raw[:, RS + c0:RS + c0 + cw],
                                     start=True, stop=True)
                tmp1 = tsb.tile([128, RL], F32, tag="tmp1")
                nc.vector.tensor_mul(tmp1[:], p_rot[:], t_sin[:])
                # in-place: dst *= cos (WAR vs the rot-matmul read), then += tmp1
                # (on GpSimd: SBUF-only operands, frees the DVE for the psum ops)
                nc.gpsimd.tensor_mul(dst[:, RS:RE], raw[:, RS:RE], t_cos[:])
                nc.vector.tensor_add(dst[:, RS:RE], dst[:, RS:RE], tmp1[:])
                # det region: multiply by the temperature pattern (psum operand)
                nc.vector.tensor_mul(dst[:, DS:DE], raw[:, DS:DE], p_tem[:])
                if raw is not dst:
                    nc.vector.tensor_copy(dst[:, 0:RS], raw[:, 0:RS])
                    if SP > DE:
                        nc.vector.tensor_copy(dst[:, DE:SP], raw[:, DE:SP])

            # ---- qT: out[dout_tile, s] ----
            for m in range(KT):
                for nb0 in range(0, SP, SQB):
                    p = ps_q.tile([128, SQB], F32, tag="qp")
                    for k in range(KT):
                        nc.tensor.matmul(p[:], qw[k][:, m * 128:(m + 1) * 128],
                                         t_hsT[k][:, nb0:nb0 + SQB],
                                         start=(k == 0), stop=(k == KT - 1))
                    nc.scalar.activation(t_qTf[m][:, nb0:nb0 + SQB], p[:], AF.Identity,
                                         bias=t_qb[:, m:m + 1], scale=cfg.HD ** -0.5)
                if m > 0:
                    finalize_qk(t_qTf[m - 1], t_qTf[m - 1])

            # ---- kT ----
            kw = [load_w(d_kwT, k) for k in range(KT)]
            # gate DMAs: after the k weights, well before attention needs them
            for t in range(NT):
                nc.sync.dma_start(t_gate[t][:], d_gateT[t * 128:(t + 1) * 128, :])
            for m in range(KT):
                for nb0 in range(0, SP, SQB):
                    p = ps_q.tile([128, SQB], F32, tag="qp", name="kp")
                    for k in range(KT):
                        nc.tensor.matmul(p[:], kw[k][:, m * 128:(m + 1) * 128],
                                         t_hsT[k][:, nb0:nb0 + SQB],
                                         start=(k == 0), stop=(k == KT - 1))
                    nc.scalar.copy(t_kTf[m][:, nb0:nb0 + SQB], p[:])
                if m == 0:
                    finalize_qk(t_qTf[KT - 1], t_qTf[KT - 1])
                if m > 0:
                    finalize_qk(t_kTf[m - 1], t_kTf[m - 1])
            finalize_qk(t_kTf[KT - 1], t_kTf[KT - 1])

            # ---- v: out[s_tile, dout] interleaved with a ones column per head
            vw = [load_w(d_vwT, k) for k in range(KT)]
            t_vb = sb1.tile([1, cfg.D], F32R, tag="vb")
            nc.sync.dma_start(t_vb[:], d_vwT[cfg.D:cfg.D + 1, :])
            t_onescol = sb1.tile([128, H], F32R, tag="onescol")
            nc.sync.dma_start(t_onescol[:], d_onescol[:, :])
            for mt in range(NT):
                p = ps_big.tile([128, cfg.D], F32, tag="big")
                for n0 in range(0, cfg.D, 512):
                    nw = min(512, cfg.D - n0)
                    nc.tensor.matmul(p[:, n0:n0 + nw],
                                     t_ones65[0:1, :],
                                     t_vb[0:1, n0:n0 + nw],
                                     start=True, stop=False)
                    for k in range(KT):
                        nc.tensor.matmul(p[:, n0:n0 + nw],
                                         t_hsT[k][:, mt * 128:(mt + 1) * 128],
                                         vw[k][:, n0:n0 + nw],
                                         start=False, stop=(k == KT - 1))
                vin = p[:, :].rearrange("p (h j) -> p h j", h=H)
                v3 = t_v[mt][:, :].rearrange("p (h j) -> p h j", j=65)
                nc.scalar.activation(v3[:, :, 0:HD], vin, AF.Identity,
                                     bias=0.0, scale=1.0)
                oc3 = t_onescol[:, :].rearrange("p (h o) -> p h o", o=1)
                nc.vector.tensor_copy(v3[:, :, HD:65], oc3)

        # ==================== phase 3: attention + output projection ========
        with ExitStack() as p3:
          if not cfg.ONLY_PHASE1:
              sb3 = p3.enter_context(tc.tile_pool(name="sb3", bufs=1))
              esb = p3.enter_context(tc.tile_pool(name="esb", bufs=8))
              csb = p3.enter_context(tc.tile_pool(name="csb", bufs=2))
              ps_sc = p3.enter_context(tc.tile_pool(name="ps_sc", bufs=4, space="PSUM"))
              ps_ctx = p3.enter_context(tc.tile_pool(name="ps_ctx", bufs=4, space="PSUM"))

              t_ob = sb3.tile([128, cfg.D], F32, tag="ob")
              nc.sync.dma_start(t_ob[:], d_ob[:, :])
              # output-projection weights
              t_ow = []
              for k in range(KT):
                  w = sb3.tile([128, cfg.D], F32R, tag=f"ow{k}")
                  nc.sync.dma_start(w[:], d_owT[k * 128:(k + 1) * 128, :])
                  t_ow.append(w)

              def norm_head(h, p_ctx, t_ctxT):
                  """1/den broadcast over 64 hd rows, write into the ctxT tile."""
                  ht, hr = h // 2, (h % 2) * 64
                  t_rc = csb.tile([65, SQB], F32R, tag="recip", bufs=3, name="rc")
                  with nc.allow_low_precision(reason="f32r recip of softmax denom"):
                      nc.vector.reciprocal(t_rc[64:65, :], p_ctx[64:65, :])
                  p_bc = ps_ctx.tile([64, SQB], F32, tag="ctx", name="bc")
                  nc.tensor.matmul(p_bc[:], t_ones65[64:65, 0:64], t_rc[64:65, :],
                                   start=True, stop=True)
                  t_cu = csb.tile([64, SQB], F32, tag="cu", bufs=3, name="cu")
                  nc.vector.tensor_copy(t_cu[:], p_ctx[0:64, :])
                  if hr == 0:
                      nc.vector.tensor_mul(t_ctxT[ht][0:64, :], t_cu[:], p_bc[:])
                  else:
                      t_hc = csb.tile([64, SQB], F32R, tag="hctx", bufs=3, name="hc")
                      nc.vector.tensor_mul(t_hc[:], t_cu[:], p_bc[:])
                      nc.sync.dma_start(t_ctxT[ht][64:128, :], t_hc[:])

              for b in range(NB):
                  b0 = b * SQB
                  t_ctxT = [csb.tile([128, SQB], F32R, tag=f"ctxT{k}", name=f"ctxT{k}", bufs=3)
                            for k in range(KT)]
                  # heads processed in pairs: the two K=64 score matmuls target
                  # disjoint 64-row strips of the PE array (row tiling) and run
                  # concurrently on hardware
                  for hp in range(H // 2):
                      h0, h1 = 2 * hp, 2 * hp + 1
                      p_ctx0 = ps_ctx.tile([65, SQB], F32, tag="ctx", name="ctx0")
                      p_ctx1 = ps_ctx.tile([65, SQB], F32, tag="ctx", name="ctx1")
                      for skt in range(NT):
                          sc0 = ps_sc.tile([128, 512], F32, tag="sc", name="sc0")
                          sc1 = ps_sc.tile([128, 512], F32, tag="sc", name="sc1")
                          nc.tensor.matmul(sc0[:, 0:SQB], t_ident[:],
                                           t_gate[skt][:, b0:b0 + SQB],
                                           start=True, stop=False)
                          nc.tensor.matmul(sc1[:, 0:SQB], t_ident[:],
                                           t_gate[skt][:, b0:b0 + SQB],
                                           start=True, stop=False)
                          nc.tensor.matmul(sc0[:, 0:SQB],
                                           t_kTf[hp][0:64, skt * 128:(skt + 1) * 128],
                                           t_qTf[hp][0:64, b0:b0 + SQB],
                                           start=False, stop=True,
                                           tile_position=(0, 0))
                          nc.tensor.matmul(sc1[:, 0:SQB],
                                           t_kTf[hp][64:128, skt * 128:(skt + 1) * 128],
                                           t_qTf[hp][64:128, b0:b0 + SQB],
                                           start=False, stop=True,
                                           tile_position=(64, 0))
                          e0 = esb.tile([128, SQB], F32R, tag="exp", name="e0")
                          nc.scalar.activation(e0[:], sc0[:, 0:SQB], AF.Exp,
                                               bias=0.0, scale=1.0)
                          e1 = esb.tile([128, SQB], F32R, tag="exp", name="e1")
                          nc.scalar.activation(e1[:], sc1[:, 0:SQB], AF.Exp,
                                               bias=0.0, scale=1.0)
                          nc.tensor.matmul(p_ctx0[:], t_v[skt][:, h0 * 65:h0 * 65 + 65],
                                           e0[:], start=(skt == 0), stop=(skt == NT - 1))
                          nc.tensor.matmul(p_ctx1[:], t_v[skt][:, h1 * 65:h1 * 65 + 65],
                                           e1[:], start=(skt == 0), stop=(skt == NT - 1))
                      norm_head(h0, p_ctx0, t_ctxT)
                      norm_head(h1, p_ctx1, t_ctxT)

                  # output projection for this sq block (psum chunks share sc slots)
                  for mt in range(SQB // 128):
                      t_out = csb.tile([128, cfg.D], F32, tag="out")
                      for n0 in range(0, cfg.D, 512):
                          nw = min(512, cfg.D - n0)
                          p_o = ps_ctx.tile([128, nw], F32, tag="ctx", name="po")
                          for k in range(KT):
                              nc.tensor.matmul(p_o[:],
                                               t_ctxT[k][:, mt * 128:(mt + 1) * 128],
                                               t_ow[k][:, n0:n0 + nw],
                                               start=(k == 0), stop=(k == KT - 1))
                          nc.vector.tensor_add(t_out[:, n0:n0 + nw], p_o[:],
                                               t_ob[:, n0:n0 + nw])
                          r0 = b0 + mt * 128
                          nc.sync.dma_start(d_out[r0:r0 + 128, n0:n0 + nw],
                                            t_out[:, n0:n0 + nw])

    nc.compile()
    return nc


# ---------------- host-side prep + dispatch --------------------------------


def _host_prep(cfg: CFG, hidden_states, q_w, q_b, k_w, v_w, v_b, o_w, o_b,
               cos, sin, ph, pw, gate):
    KT, SP, H, HD = cfg.KT, cfg.SP, cfg.H, cfg.HD
    D, S = cfg.D, cfg.S
    DET = cfg.DET_END - cfg.DET_START
    half = HD // 2

    shared = {}
    shared["qwT"] = round_f32r(q_w.T)
    shared["kwT"] = round_f32r(k_w.T)
    vwT = np.zeros(((KT + 1) * 128, D), np.float32)
    vwT[:D] = v_w.T
    vwT[D] = v_b
    shared["vwT"] = round_f32r(vwT)
    shared["owT"] = round_f32r(o_w.T)
    gateT = np.zeros((SP, SP), np.float32)
    gateT[:S, :S] = gate[0, 0].T
    gateT[S:, :] = cfg.GATE_NEG
    shared["gateT"] = round_f32r(gateT)
    # biases: qb pre-scaled by 1/sqrt(hd), laid out [128, KT]
    qb = (q_b.astype(np.float32) * (HD ** -0.5)).reshape(KT, 128).T
    shared["qb"] = np.ascontiguousarray(qb)
    shared["ob"] = np.broadcast_to(o_b.astype(np.float32)[None, :], (128, D)).copy()
    # rope tables: [128, RL] = two stacked head-copies of cos/sin transposed
    cosT = cos.T.astype(np.float32)                       # [HD, RL]
    sinT = sin.T.astype(np.float32)
    shared["cosT2"] = np.vstack([cosT, cosT]).astype(np.float32)
    shared["sinT2"] = np.vstack([sinT, sinT]).astype(np.float32)
    # rotation matrix R (rotate_half along the hd partition dim), applied as
    # R @ x via lhsT = R.T; R spans two stacked heads per 128-partition tile
    R = np.zeros((128, 128), np.float32)
    for blk in range(2):
        o = blk * HD
        for j in range(half):
            R[o + j, o + half + j] = -1.0
            R[o + half + j, o + j] = 1.0
    shared["rotT"] = round_f32r(R.T)
    shared["ident"] = round_f32r(np.eye(128, dtype=np.float32))
    shared["ones65"] = round_f32r(np.ones((65, 128), np.float32))
    shared["onescol"] = round_f32r(np.ones((128, H), np.float32))
    maska = np.zeros((1, 128), np.float32)
    maskb = np.zeros((1, 128), np.float32)
    for p in range(128):
        if (p % HD) < half:
            maska[0, p] = 1.0
        else:
            maskb[0, p] = 1.0
    shared["masks"] = round_f32r(np.concatenate([maska, maskb], axis=1))
    shared["ph"] = ph.astype(np.float32).reshape(1, DET)
    shared["pw"] = pw.astype(np.float32).reshape(1, DET)

    in_maps = []
    for c in range(cfg.N_CORES):
        hsT = np.zeros((D, SP), np.float32)
        hsT[:, :S] = hidden_states[c].T
        m = dict(shared)
        m["hsT"] = round_f32r(hsT)
        in_maps.append(m)
    return in_maps


_NC_CACHE = {}


def kernel(hidden_states, q_w, q_b, k_w, v_w, v_b, o_w, o_b,
           cos, sin, ph, pw, gate,
           rope_start=5, rope_end=1029, det_start=1029, det_end=1129):
    cfg = CFG()
    in_maps = _host_prep(cfg, np.asarray(hidden_states, np.float32),
                         np.asarray(q_w, np.float32), np.asarray(q_b, np.float32),
                         np.asarray(k_w, np.float32), np.asarray(v_w, np.float32),
                         np.asarray(v_b, np.float32), np.asarray(o_w, np.float32),
                         np.asarray(o_b, np.float32), np.asarray(cos, np.float32),
                         np.asarray(sin, np.float32), np.asarray(ph, np.float32),
                         np.asarray(pw, np.float32), np.asarray(gate, np.float32))
    if "nc" not in _NC_CACHE:
        _NC_CACHE["nc"] = build_nc(cfg)
    nc = _NC_CACHE["nc"]
    res = run_bass_kernel_spmd(nc, in_maps, list(range(cfg.N_CORES)))
    out = np.stack([res.results[c]["out"][:cfg.S] for c in range(cfg.N_CORES)])
    return out.astype(np.float32)



# revision 7
# speedup vs baseline: 1.1379x; 1.1379x over previous
"""DINOv3 ViT attention (RoPE + det-temp scaling + additive gate) on 8 TRN2 cores.

Sharding: pure data-parallel over batch (B=8 -> 1 batch element per core).
Weights / gate / rope tables replicated. No collectives.

v2 design (engine budget per the TimelineSim cost model):
  - gate folded multiplicatively: EG = exp(gate^T) precomputed on host (bf16);
    after the ACT exp of raw scores, one DVE bf16 multiply applies it
    (replaces the per-head identity-matmul gate copies on PE).
  - scores per (head, skt): qT in bf16 (moving operand -> 1 cyc/col at any
    width), kT f32r (stationary), full-sq psum [128, 1152] in 3 chunks.
  - exp: ONE wide ACT op per (head, skt) over [128, 1129] (amortizes the
    ~185ns per-op access-latency penalty).
  - ctx flipped: out[sq_tile, 65] = e'^T @ v  (lhsT = e' bf16), N=65 per
    matmul -> half the PE columns of the [hd, sq] orientation; the v ones
    column lands the softmax denominator as a per-partition column, so
    normalization is one strided DVE reciprocal + 9 GpSimd scalar muls.
  - ctx [sq, hd] bf16 -> ctxT [hd, sq] via DMA XBAR transposes (idle engine).
  - output projection from ctxT bf16 (lhsT) x owT f32r, tail after attention.
"""
import numpy as np
from contextlib import ExitStack

import ml_dtypes
import concourse.bacc as bacc
import concourse.mybir as mybir
import concourse.tile as tile
from concourse.bass_utils import run_bass_kernel_spmd

F32 = mybir.dt.float32
F32R = mybir.dt.float32r
BF16 = mybir.dt.bfloat16
AF = mybir.ActivationFunctionType

# ---------------- problem config (hardcoded per harness contract) ------------


class CFG:
    B = 8
    S = 1129
    SP = 1152            # padded S (9 * 128)
    D = 768
    H = 12
    HD = 64
    ROPE_START = 5
    ROPE_END = 1029
    DET_START = 1029
    DET_END = 1129
    P_SCALE = 2.0
    N_CORES = 8
    SQB = 384            # projection eviction chunk
    GATE_NEG = -30.0     # gate value for pad keys: exp(-30) ~ 9e-14
    CTX_STRIDE = 74      # ctx psum window stride (65-wide windows, no
                         # 512-col psum bank crossings for 9 windows)

    @property
    def KT(self):
        return self.D // 128          # dout/din 128-tiles (6)

    @property
    def NT(self):
        return self.SP // 128         # s 128-tiles (9)

    @property
    def ROPE_LEN(self):
        return self.ROPE_END - self.ROPE_START


def round_f32r(x: np.ndarray) -> np.ndarray:
    """Round fp32 to the fp32r format (11 mantissa bits, RNE)."""
    b = np.ascontiguousarray(x, dtype=np.float32).view(np.uint32)
    low = b & np.uint32(0xFFF)
    b = b & np.uint32(0xFFFFF000)
    rnd = (low > 0x800) | ((low == 0x800) & (((b >> 12) & 1) != 0))
    b = b + (rnd.astype(np.uint32) << 12)
    return b.view(np.float32)


def to_bf16(x: np.ndarray) -> np.ndarray:
    return np.ascontiguousarray(x, dtype=np.float32).astype(ml_dtypes.bfloat16)


# ---------------- device program ------------------------------------------


def build_nc(cfg: CFG):
    nc = bacc.Bacc(trn_type="TRN2", target_bir_lowering=False, debug=False)
    KT, NT, SQB, SP = cfg.KT, cfg.NT, cfg.SQB, cfg.SP
    H, HD = cfg.H, cfg.HD
    RS, RE, DS, DE = cfg.ROPE_START, cfg.ROPE_END, cfg.DET_START, cfg.DET_END
    RL = cfg.ROPE_LEN
    DET = DE - DS
    S = cfg.S
    CS = cfg.CTX_STRIDE

    # ---- dram parameters (per core) ----
    d_hsT = nc.dram_tensor("hsT", [cfg.D, SP], F32R, kind="ExternalInput").ap()
    d_qwT = nc.dram_tensor("qwT", [cfg.D, cfg.D], F32R, kind="ExternalInput").ap()
    d_kwT = nc.dram_tensor("kwT", [cfg.D, cfg.D], F32R, kind="ExternalInput").ap()
    d_vwT = nc.dram_tensor("vwT", [(KT + 1) * 128, cfg.D], F32R, kind="ExternalInput").ap()
    d_owT = nc.dram_tensor("owT", [cfg.D, cfg.D], F32R, kind="ExternalInput").ap()
    d_EG = nc.dram_tensor("EG", [SP, SP], BF16, kind="ExternalInput").ap()
    d_qb = nc.dram_tensor("qb", [128, KT], F32, kind="ExternalInput").ap()
    d_ob = nc.dram_tensor("ob", [128, cfg.D], F32, kind="ExternalInput").ap()
    d_cosT2 = nc.dram_tensor("cosT2", [128, RL], F32, kind="ExternalInput").ap()
    d_sinT2 = nc.dram_tensor("sinT2", [128, RL], F32, kind="ExternalInput").ap()
    d_rotT = nc.dram_tensor("rotT", [128, 128], F32R, kind="ExternalInput").ap()
    d_ones = nc.dram_tensor("ones", [1, 128], F32R, kind="ExternalInput").ap()
    d_onescol = nc.dram_tensor("onescol", [128, H], BF16, kind="ExternalInput").ap()
    d_masks = nc.dram_tensor("masks", [1, 256], F32R, kind="ExternalInput").ap()
    d_ph = nc.dram_tensor("ph", [1, DET], F32, kind="ExternalInput").ap()
    d_pw = nc.dram_tensor("pw", [1, DET], F32, kind="ExternalInput").ap()
    d_out = nc.dram_tensor("out", [SP, cfg.D], F32, kind="ExternalOutput").ap()

    with tile.TileContext(nc) as tc, ExitStack() as gctx:
        # ---------------- global sbuf (spans the whole kernel) --------------
        gsb = gctx.enter_context(tc.tile_pool(name="gsb", bufs=1))

        t_ones = gsb.tile([1, 128], F32R, tag="ones")
        nc.sync.dma_start(t_ones[:], d_ones[:, :])

        # persistent activations
        t_qTf = [gsb.tile([128, SP], BF16, tag=f"qTf{m}", name=f"qTf{m}") for m in range(KT)]
        t_kTf = [gsb.tile([128, SP], F32R, tag=f"kTf{m}", name=f"kTf{m}") for m in range(KT)]
        t_v = [gsb.tile([128, H * 65], BF16, tag=f"v{t}", name=f"v{t}") for t in range(NT)]
        # exp(gate^T) tiles; DMAs emitted later (after phase-1 critical loads)
        t_EG = [gsb.tile([128, SP], BF16, tag=f"eg{t}", name=f"eg{t}") for t in range(NT)]
        # gated-exp ring
        NE = 4
        t_e = [gsb.tile([128, SP], BF16, tag=f"e{i}", name=f"e{i}") for i in range(NE)]
        # normalized context [sq, hd] and transposed [hd, sq]
        t_ctx = [gsb.tile([128, cfg.D], BF16, tag=f"ctx{t}", name=f"ctx{t}") for t in range(NT)]
        t_ctxT = [gsb.tile([128, SP], BF16, tag=f"ctxT{k}", name=f"ctxT{k}") for k in range(KT)]
        t_rec = [gsb.tile([128, NT], F32, tag=f"rec{i}", name=f"rec{i}") for i in range(2)]

        # ==================== phase 1: projections + RoPE/det ===============
        with ExitStack() as p1:
            sb1 = p1.enter_context(tc.tile_pool(name="sb1", bufs=1))
            wsb = p1.enter_context(tc.tile_pool(name="wsb", bufs=2))
            ps_q = p1.enter_context(tc.tile_pool(name="ps_q", bufs=2, space="PSUM"))
            ps_big = p1.enter_context(tc.tile_pool(name="ps_big", bufs=2, space="PSUM"))
            ps_tem = p1.enter_context(tc.tile_pool(name="ps_tem", bufs=1, space="PSUM"))

            # hsT tiles interleaved with qw loads so the first q chain can
            # start accumulating as tiles arrive
            t_hsT = []
            qw = []
            for k in range(KT):
                t = sb1.tile([128, SP], F32R, tag=f"hsT{k}", name=f"hsT{k}")
                eng = nc.sync if k % 2 == 0 else nc.scalar
                eng.dma_start(t[:], d_hsT[k * 128:(k + 1) * 128, :])
                t_hsT.append(t)
                w = wsb.tile([128, cfg.D], F32R, tag=f"w{k}", name=f"qw{k}")
                weng = nc.scalar if k % 2 == 0 else nc.sync
                weng.dma_start(w[:], d_qwT[k * 128:(k + 1) * 128, :])
                qw.append(w)

            # rope/det tables
            t_cos = sb1.tile([128, RL], F32, tag="cos")
            nc.scalar.dma_start(t_cos[:], d_cosT2[:, :])
            t_sin = sb1.tile([128, RL], F32, tag="sin")
            nc.scalar.dma_start(t_sin[:], d_sinT2[:, :])
            t_rotT = sb1.tile([128, 128], F32R, tag="rotT")
            nc.sync.dma_start(t_rotT[:], d_rotT[:, :])
            t_qb = sb1.tile([128, KT], F32, tag="qb")
            nc.sync.dma_start(t_qb[:], d_qb[:, :])

            # det temperature pattern tile: [128, DET] via two outer products
            t_ms = sb1.tile([1, 256], F32R, tag="ms")
            nc.sync.dma_start(t_ms[:], d_masks[:, :])
            t_ph = sb1.tile([1, DET], F32, tag="ph")
            nc.sync.dma_start(t_ph[:], d_ph[:, :])
            t_pw = sb1.tile([1, DET], F32, tag="pw")
            nc.sync.dma_start(t_pw[:], d_pw[:, :])
            t_eh = sb1.tile([1, DET], F32R, tag="eh")
            nc.scalar.activation(t_eh[:], t_ph[:], AF.Exp, bias=0.0, scale=cfg.P_SCALE)
            t_ew = sb1.tile([1, DET], F32R, tag="ew")
            nc.scalar.activation(t_ew[:], t_pw[:], AF.Exp, bias=0.0, scale=cfg.P_SCALE)
            p_tem = ps_tem.tile([128, DET], F32, tag="tem")
            nc.tensor.matmul(p_tem[:], t_ms[0:1, 0:128], t_eh[:], start=True, stop=False)
            nc.tensor.matmul(p_tem[:], t_ms[0:1, 128:256], t_ew[:], start=False, stop=True)

            def load_w(dram, k, nm):
                t = wsb.tile([128, cfg.D], F32R, tag=f"w{k}", name=nm)
                nc.sync.dma_start(t[:], dram[k * 128:(k + 1) * 128, :])
                return t

            def finalize_qk(dst):
                """RoPE + det-temp in place on dst [128, SP] (q bf16 / k f32r).

                dst[RS:RE] = dst*cos + rot(dst)*sin ; dst[DS:DE] *= tem.
                rot via PE permutation matmul (psum), cos-mul on GpSimd,
                sin-mul + add on DVE, det-mul on GpSimd.
                """
                p_rot = ps_big.tile([128, RL], F32, tag="big", name="rot")
                for c0 in range(0, RL, 512):
                    nc.tensor.matmul(p_rot[:, c0:c0 + 512], t_rotT[:],
                                     dst[:, RS + c0:RS + c0 + 512],
                                     start=True, stop=True)
                tmp1 = sb1.tile([128, RL], F32, tag="tmp1", bufs=2)
                nc.vector.tensor_mul(tmp1[:], p_rot[:], t_sin[:])
                nc.gpsimd.tensor_mul(dst[:, RS:RE], dst[:, RS:RE], t_cos[:])
                nc.vector.tensor_add(dst[:, RS:RE], dst[:, RS:RE], tmp1[:])
                nc.gpsimd.tensor_mul(dst[:, DS:DE], dst[:, DS:DE], p_tem[:])

            # ---- qT: out[dout_tile, s] in bf16, bias via DVE eviction ----
            for m in range(KT):
                for nb0 in range(0, SP, SQB):
                    p = ps_q.tile([128, SQB], F32, tag="qp")
                    for k in range(KT):
                        nc.tensor.matmul(p[:], qw[k][:, m * 128:(m + 1) * 128],
                                         t_hsT[k][:, nb0:nb0 + SQB],
                                         start=(k == 0), stop=(k == KT - 1))
                    nc.vector.tensor_scalar_add(t_qTf[m][:, nb0:nb0 + SQB], p[:],
                                                t_qb[:, m:m + 1])
                if m > 0:
                    finalize_qk(t_qTf[m - 1])

            # ---- kT (f32r, no bias) ----
            kw = [load_w(d_kwT, k, f"kw{k}") for k in range(KT)]
            # EG DMAs: after the k weights, well before attention needs them
            for t in range(NT):
                nc.sync.dma_start(t_EG[t][:], d_EG[t * 128:(t + 1) * 128, :])
            for m in range(KT):
                for nb0 in range(0, SP, SQB):
                    p = ps_q.tile([128, SQB], F32, tag="qp", name="kp")
                    for k in range(KT):
                        nc.tensor.matmul(p[:], kw[k][:, m * 128:(m + 1) * 128],
                                         t_hsT[k][:, nb0:nb0 + SQB],
                                         start=(k == 0), stop=(k == KT - 1))
                    nc.scalar.copy(t_kTf[m][:, nb0:nb0 + SQB], p[:])
                if m == 0:
                    finalize_qk(t_qTf[KT - 1])
                if m > 0:
                    finalize_qk(t_kTf[m - 1])
            finalize_qk(t_kTf[KT - 1])

            # ---- v: out[s_tile, dout] bf16, interleaved ones column per head
            vw = [load_w(d_vwT, k, f"vw{k}") for k in range(KT)]
            t_vb = sb1.tile([1, cfg.D], F32R, tag="vb")
            nc.sync.dma_start(t_vb[:], d_vwT[cfg.D:cfg.D + 1, :])
            t_onescol = sb1.tile([128, H], BF16, tag="onescol")
            nc.sync.dma_start(t_onescol[:], d_onescol[:, :])
            for mt in range(NT):
                p = ps_big.tile([128, cfg.D], F32, tag="big", name="vp")
                for n0 in range(0, cfg.D, 512):
                    nw = min(512, cfg.D - n0)
                    nc.tensor.matmul(p[:, n0:n0 + nw],
                                     t_ones[0:1, :],
                                     t_vb[0:1, n0:n0 + nw],
                                     start=True, stop=False)
                    for k in range(KT):
                        nc.tensor.matmul(p[:, n0:n0 + nw],
                                         t_hsT[k][:, mt * 128:(mt + 1) * 128],
                                         vw[k][:, n0:n0 + nw],
                                         start=False, stop=(k == KT - 1))
                vin = p[:, :].rearrange("p (h j) -> p h j", h=H)
                v3 = t_v[mt][:, :].rearrange("p (h j) -> p h j", j=65)
                nc.scalar.activation(v3[:, :, 0:HD], vin, AF.Identity,
                                     bias=0.0, scale=1.0)
                oc3 = t_onescol[:, :].rearrange("p (h o) -> p h o", o=1)
                nc.vector.tensor_copy(v3[:, :, HD:65], oc3)

        # ==================== phase 3: attention + tail ======================
        # psum: 2 x scores (3 banks each) + ctx (2 banks) = 8 banks
        with ExitStack() as p34:
          sb3 = p34.enter_context(tc.tile_pool(name="sb3", bufs=1))
          t_ob = sb3.tile([128, cfg.D], F32, tag="ob")
          t_ow = [sb3.tile([128, cfg.D], F32R, tag=f"ow{k}", name=f"ow{k}")
                  for k in range(KT)]
          with ExitStack() as p3:
            ps_sc = p3.enter_context(tc.tile_pool(name="ps_sc", bufs=2, space="PSUM"))
            ps_ctx = p3.enter_context(tc.tile_pool(name="ps_ctx", bufs=1, space="PSUM"))

            nc.sync.dma_start(t_ob[:], d_ob[:, :])
            for k in range(KT):
                nc.sync.dma_start(t_ow[k][:], d_owT[k * 128:(k + 1) * 128, :])

            # zero the pad columns of the e-ring once; exp/gate writes cover
            # only [0:S], ctx matmuls read [1024:1152] for the last sq tile
            for i in range(NE):
                nc.gpsimd.memset(t_e[i][:, S:SP], 0.0)

            ei = 0
            for h in range(H):
                hp, hr = h // 2, (h % 2) * 64
                p_ctx = ps_ctx.tile([128, NT * CS], F32, tag="ctx")
                ctx3 = p_ctx[:, :].rearrange("p (m w) -> p m w", w=CS)
                for skt in range(NT):
                    sc = ps_sc.tile([128, SP], F32, tag="sc")
                    for c0, cw in ((0, 512), (512, 512), (1024, 128)):
                        nc.tensor.matmul(sc[:, c0:c0 + cw],
                                         t_kTf[hp][hr:hr + 64, skt * 128:(skt + 1) * 128],
                                         t_qTf[hp][hr:hr + 64, c0:c0 + cw],
                                         start=True, stop=True)
                    e = t_e[ei % NE]
                    ei += 1
                    nc.scalar.activation(e[:, 0:S], sc[:, 0:S], AF.Exp,
                                         bias=0.0, scale=1.0)
                    nc.vector.tensor_mul(e[:, 0:S], e[:, 0:S], t_EG[skt][:, 0:S])
                    for mt in range(NT):
                        nc.tensor.matmul(ctx3[:, mt, 0:65],
                                         e[:, mt * 128:(mt + 1) * 128],
                                         t_v[skt][:, h * 65:h * 65 + 65],
                                         start=(skt == 0), stop=(skt == NT - 1))
                # normalization: denominators are column 64 of each window
                rec = t_rec[h % 2]
                r3 = rec[:, :].rearrange("p (m o) -> p m o", o=1)
                nc.vector.reciprocal(r3[:, :, :], ctx3[:, :, 64:65])
                for mt in range(NT):
                    nc.gpsimd.tensor_scalar_mul(
                        t_ctx[mt][:, h * HD:(h + 1) * HD],
                        ctx3[:, mt, 0:HD], rec[:, mt:mt + 1])
                if h % 2 == 1:
                    for mt in range(NT):
                        nc.sync.dma_start_transpose(
                            t_ctxT[hp][:, mt * 128:(mt + 1) * 128],
                            t_ctx[mt][:, hp * 128:(hp + 1) * 128])

          # ==================== tail: output projection =====================
          with ExitStack() as p4:
            sb4 = p4.enter_context(tc.tile_pool(name="sb4", bufs=1))
            ps_o = p4.enter_context(tc.tile_pool(name="ps_o", bufs=3, space="PSUM"))
            for mt in range(NT):
                t_out = sb4.tile([128, cfg.D], F32, tag="out", bufs=3)
                p_o = ps_o.tile([128, cfg.D], F32, tag="po")
                for n0 in range(0, cfg.D, 512):
                    nw = min(512, cfg.D - n0)
                    for k in range(KT):
                        nc.tensor.matmul(p_o[:, n0:n0 + nw],
                                         t_ctxT[k][:, mt * 128:(mt + 1) * 128],
                                         t_ow[k][:, n0:n0 + nw],
                                         start=(k == 0), stop=(k == KT - 1))
                nc.gpsimd.tensor_add(t_out[:], p_o[:], t_ob[:])
                nc.sync.dma_start(d_out[mt * 128:(mt + 1) * 128, :], t_out[:])

    nc.compile()
    return nc


# ---------------- host-side prep + dispatch --------------------------------


def _host_prep(cfg: CFG, hidden_states, q_w, q_b, k_w, v_w, v_b, o_w, o_b,
               cos, sin, ph, pw, gate):
    KT, SP, H, HD = cfg.KT, cfg.SP, cfg.H, cfg.HD
    D, S = cfg.D, cfg.S
    DET = cfg.DET_END - cfg.DET_START
    half = HD // 2
    scale = HD ** -0.5

    shared = {}
    shared["qwT"] = round_f32r(q_w.T * scale)
    shared["kwT"] = round_f32r(k_w.T)
    vwT = np.zeros(((KT + 1) * 128, D), np.float32)
    vwT[:D] = v_w.T
    vwT[D] = v_b
    shared["vwT"] = round_f32r(vwT)
    shared["owT"] = round_f32r(o_w.T)
    # multiplicative gate: exp(gate^T), pad keys killed via exp(GATE_NEG)
    gateT = np.full((SP, SP), cfg.GATE_NEG, np.float32)
    gateT[:S, :S] = gate[0, 0].T
    shared["EG"] = to_bf16(np.exp(gateT))
    # q bias pre-scaled, laid out [128, KT]
    qb = (q_b.astype(np.float32) * scale).reshape(KT, 128).T
    shared["qb"] = np.ascontiguousarray(qb)
    shared["ob"] = np.broadcast_to(o_b.astype(np.float32)[None, :], (128, D)).copy()
    # rope tables: [128, RL] = two stacked head-copies of cos/sin transposed
    cosT = cos.T.astype(np.float32)                       # [HD, RL]
    sinT = sin.T.astype(np.float32)
    shared["cosT2"] = np.vstack([cosT, cosT]).astype(np.float32)
    shared["sinT2"] = np.vstack([sinT, sinT]).astype(np.float32)
    # rotation matrix R (rotate_half along the hd partition dim), applied as
    # R @ x via lhsT = R.T; R spans two stacked heads per 128-partition tile
    R = np.zeros((128, 128), np.float32)
    for blk in range(2):
        o = blk * HD
        for j in range(half):
            R[o + j, o + half + j] = -1.0
            R[o + half + j, o + j] = 1.0
    shared["rotT"] = round_f32r(R.T)
    shared["ones"] = round_f32r(np.ones((1, 128), np.float32))
    shared["onescol"] = to_bf16(np.ones((128, H), np.float32))
    maska = np.zeros((1, 128), np.float32)
    maskb = np.zeros((1, 128), np.float32)
    for p in range(128):
        if (p % HD) < half:
            maska[0, p] = 1.0
        else:
            maskb[0, p] = 1.0
    shared["masks"] = round_f32r(np.concatenate([maska, maskb], axis=1))
    shared["ph"] = ph.astype(np.float32).reshape(1, DET)
    shared["pw"] = pw.astype(np.float32).reshape(1, DET)

    in_maps = []
    for c in range(cfg.N_CORES):
        hsT = np.zeros((D, SP), np.float32)
        hsT[:, :S] = hidden_states[c].T
        m = dict(shared)
        m["hsT"] = round_f32r(hsT)
        in_maps.append(m)
    return in_maps


_NC_CACHE = {}


def kernel(hidden_states, q_w, q_b, k_w, v_w, v_b, o_w, o_b,
           cos, sin, ph, pw, gate,
           rope_start=5, rope_end=1029, det_start=1029, det_end=1129):
    cfg = CFG()
    in_maps = _host_prep(cfg, np.asarray(hidden_states, np.float32),
                         np.asarray(q_w, np.float32), np.asarray(q_b, np.float32),
                         np.asarray(k_w, np.float32), np.asarray(v_w, np.float32),
                         np.asarray(v_b, np.float32), np.asarray(o_w, np.float32),
                         np.asarray(o_b, np.float32), np.asarray(cos, np.float32),
                         np.asarray(sin, np.float32), np.asarray(ph, np.float32),
                         np.asarray(pw, np.float32), np.asarray(gate, np.float32))
    if "nc" not in _NC_CACHE:
        _NC_CACHE["nc"] = build_nc(cfg)
    nc = _NC_CACHE["nc"]
    res = run_bass_kernel_spmd(nc, in_maps, list(range(cfg.N_CORES)))
    out = np.stack([res.results[c]["out"][:cfg.S] for c in range(cfg.N_CORES)])
    return out.astype(np.float32)
